# revision 2
# baseline (speedup 1.0000x reference)
"""Trainium2 Bass kernel v4: fp16 compute, 3-engine balance, 2-way column
block split, STT-free Pool work.

Key structure:
  - 8 cores x [128 x 1024] cells + G=32 ghost columns per side; no halo
    exchange (ghosts absorb the 32-step stencil growth); shrinking valid
    windows replace ghost sanitization.
  - fp16 compute tiles (DVE 2x_1p); fp32 state, reciprocals, dt chain.
  - Two column blocks with a seam that drifts left one column per step:
    cross-step deps stay within a block, seam deps point only L -> R.
    Block op streams are round-robin merged so each in-order engine queue
    alternates blocks and a stalled op is followed by ready work.
  - Pool (GpSimd) gets only plain TensorTensor ops (neuronxcc rejects
    TensorScalarPtr on Pool).  All former scalar_tensor_tensor fusions are
    decomposed into ACT scalar-scale ops + plain TTs; 1/(2c^2) comes
    straight from recip(2c^2).
  - The dt AllReduce is launched at a merge boundary (after all B(L) Pool
    work) and finished after B(R), so the collective never blocks the Pool
    queue head.
"""

import numpy as np

import concourse.bass as bass
import concourse.bacc as bacc
import concourse.tile as tile
import concourse.mybir as mybir
from concourse import bass_isa
from concourse.bass_utils import run_bass_kernel_spmd

F32 = mybir.dt.float32
F16 = mybir.dt.float16
U8 = mybir.dt.uint8
ALU = mybir.AluOpType
ACTF = mybir.ActivationFunctionType

GAMMA = 1.4
CFL = 0.5
DX = 1e-3

NX = 1048576
NC = 8
P = 128
FPC = NX // NC // P          # 1024 cells per partition
G = 32                       # ghost width per side (>= n_steps)
W = FPC + 2 * G              # 1088 columns per partition
M = W // 2                   # block seam at step 0 (drifts left)

_CACHE = {}
_last_results = None

# tunables: engine/structure toggles swept via sim
CFG = {
    'allgather': True,     # AllGather [1,1] instead of AllReduce [P,1]
    'u_via_rinv16': True,  # ACT-copy rinv to fp16; u16 = mu16*rinv16 on DVE
    'wsc_pool': False,     # wsc = au+cc on Pool
    'den32_pool': False,   # den32 / d32 fp32 sums on Pool
    'upd_split': False,    # STT update -> TS premult (DVE) + Pool add
    'dphi_pool': True,     # flux diffs on Pool (slack: upds wait on dt)
    'drho_pool': False,    # drho on Pool
    'reduce_pool': False,  # wsc reduce on Pool
}


def _build(n_steps: int):
    nc = bacc.Bacc("TRN2", target_bir_lowering=False, debug=False,
                   enable_asserts=False, num_devices=NC)

    rho_in = nc.dram_tensor("rho_in", [P, W], F32, kind="ExternalInput")
    mu_in = nc.dram_tensor("mu_in", [P, W], F32, kind="ExternalInput")
    E_in = nc.dram_tensor("E_in", [P, W], F32, kind="ExternalInput")
    tf_in = nc.dram_tensor("tf_in", [1, 1], F32, kind="ExternalInput")
    mskL_in = nc.dram_tensor("mskL_in", [P, G], U8, kind="ExternalInput")
    mskR_in = nc.dram_tensor("mskR_in", [P, G], U8, kind="ExternalInput")
    rho_out = nc.dram_tensor("rho_out", [P, FPC], F32, kind="ExternalOutput")
    u_out = nc.dram_tensor("u_out", [P, FPC], F32, kind="ExternalOutput")
    p_out = nc.dram_tensor("p_out", [P, FPC], F32, kind="ExternalOutput")

    with tile.TileContext(nc) as tc:
        with (
            tc.tile_pool(name="sb", bufs=1) as sb,
            tc.tile_pool(name="dram", bufs=1, space="DRAM") as dram,
        ):
            rho = sb.tile([P, W], F32, tag="rho", name="rho")
            mu = sb.tile([P, W], F32, tag="mu", name="mu")
            En = sb.tile([P, W], F32, tag="En", name="En")

            N32 = 8
            N16 = 44
            wk32 = [sb.tile([P, W], F32, tag=f"w32_{i}", name=f"w32_{i}")
                    for i in range(N32)]
            wk16 = [sb.tile([P, W], F16, tag=f"w16_{i}", name=f"w16_{i}")
                    for i in range(N16)]
            free32 = list(wk32)
            free16 = list(wk16)
            live = {}
            pending = []
            defer = [False]
            def g32(name):
                t = free32.pop()
                live[name] = (t, free32)
                return t

            def g16(name):
                t = free16.pop()
                live[name] = (t, free16)
                return t

            def rel(*names):
                # Tiles freed while constructing the FIRST stream of a merge
                # pair must not be re-allocated by the second stream (their
                # thunks interleave at emission), so those parks go to
                # `pending` until the merge completes.
                for n in names:
                    t, pool = live.pop(n)
                    if defer[0]:
                        pending.append((t, pool))
                    else:
                        pool.append(t)

            def flush_pending():
                for t, pool in pending:
                    pool.append(t)
                pending.clear()

            mskL = sb.tile([P, G], U8, tag="mskL", name="mskL")
            mskR = sb.tile([P, G], U8, tag="mskR", name="mskR")
            wmaxL = sb.tile([P, 1], F32, tag="wmaxL", name="wmaxL")
            wmaxR = sb.tile([P, 1], F32, tag="wmaxR", name="wmaxR")
            wmax = sb.tile([P, 1], F32, tag="wmax", name="wmax")
            gpp = sb.tile([P, 1], F32, tag="gpp", name="gpp")
            gball = sb.tile([P, 1], F32, tag="gball", name="gball")
            gb2 = sb.tile([P, 1], F32, tag="gb2", name="gb2")
            rgi = sb.tile([P, 1], F32, tag="rgi", name="rgi")
            dt0 = sb.tile([P, 1], F32, tag="dt0", name="dt0")
            rem = sb.tile([P, 1], F32, tag="rem", name="rem")
            dtt = sb.tile([P, 1], F32, tag="dtt", name="dtt")
            tcur = sb.tile([P, 1], F32, tag="tcur", name="tcur")
            hdtn = sb.tile([P, 1], F32, tag="hdtn", name="hdtn")
            tf1 = sb.tile([1, 1], F32, tag="tf1", name="tf1")
            tfb = sb.tile([P, 1], F32, tag="tfb", name="tfb")

            cc_in = [dram.tile([1, 1], F32, tag=f"cc_in{i}",
                                name=f"cc_in{i}") for i in range(2)]
            cc_out = [dram.tile([1, 8], F32, tag=f"cc_out{i}",
                                 name=f"cc_out{i}") for i in range(2)]
            gppb = [sb.tile([1, 8], F32, tag=f"gpp{i}", name=f"gpp{i}")
                    for i in range(2)]
            g1 = sb.tile([1, 1], F32, tag="g1", name="g1")

            vec = nc.vector
            act = nc.scalar
            gps = nc.gpsimd

            # full-width shared flux tiles (the update diff at the seam
            # reads across blocks)
            Pr = sb.tile([P, W], F16, tag="Pr", name="Pr")
            Pm = sb.tile([P, W], F16, tag="Pm", name="Pm")
            Pe = sb.tile([P, W], F16, tag="Pe", name="Pe")

            # ---- prologue ----
            nc.sync.dma_start(out=rho[:], in_=rho_in.ap())
            nc.sync.dma_start(out=mu[:], in_=mu_in.ap())
            nc.sync.dma_start(out=En[:], in_=E_in.ap())
            nc.sync.dma_start(out=mskL[:], in_=mskL_in.ap())
            nc.sync.dma_start(out=mskR[:], in_=mskR_in.ap())
            nc.sync.dma_start(out=tf1[:], in_=tf_in.ap())
            gps.partition_broadcast(tfb[:], tf1[:])
            vec.memset(tcur[:], 0.0)

            def nm(base, blk):
                return f"{base}@{blk}"

            def emit_clamps(s, blk):
                ops = []
                if s == 0 or s >= G:
                    return ops
                gw = G - s
                if blk == 0:
                    for st in (rho, mu, En):
                        ops.append(lambda st=st: vec.copy_predicated(
                            st[:, s:G], mskL[:, 0:gw],
                            st[:, G:G + 1].broadcast_to((P, gw))))
                else:
                    for st in (rho, mu, En):
                        ops.append(lambda st=st: vec.copy_predicated(
                            st[:, W - G:W - s], mskR[:, 0:gw],
                            st[:, W - G - 1:W - G].broadcast_to((P, gw))))
                return ops

            # Block ranges (seam at M - s):
            #   A0 = [s, M-s)        A1 = [M-s, W-s)
            #   I0 = [s, M-s-1)      I1 = [M-s-1, W-s-1)
            #   U0 = [s+1, M-s-1)    U1 = [M-s-1, W-s-1)
            def emit_A(s, blk):
                # block R overlaps one column left of the seam so its B-phase
                # reads only its own stage-A tiles
                s = min(s, G - 1)
                lo = s if blk == 0 else M - s - 1
                hi = M - s if blk == 0 else W - s

                def A(t):
                    return t[:, lo:hi]

                ops = []
                E = ops.append

                rho16 = g16(nm("rho16", blk))
                mu16 = g16(nm("mu16", blk))
                En16 = g16(nm("En16", blk))
                E(lambda: act.copy(A(rho16), A(rho)))
                E(lambda: act.copy(A(mu16), A(mu)))
                rinv = g32(nm("rinv", blk))
                E(lambda: vec.reciprocal_approx_fast(A(rinv), A(rho)))
                rinv16 = g16(nm("rinv16", blk))
                E(lambda: act.copy(A(rinv16), A(rinv)))
                u16 = g16(nm("u16", blk))
                E(lambda: vec.tensor_tensor(A(u16), A(mu16), A(rinv16),
                                            ALU.mult))
                sq16 = g16(nm("sq16", blk))
                E(lambda: act.activation(A(sq16), A(rho), ACTF.Sqrt))
                E4 = g16(nm("E4", blk))
                E(lambda: act.activation(A(E4), A(En), ACTF.Copy, scale=0.4))
                E(lambda: act.copy(A(En16), A(En)))
                # p = E4 - 0.2*q, Fm = E4 + 0.8*q  via q5 = mu*(0.2 u)
                u5 = g16(nm("u5", blk))
                E(lambda: act.activation(A(u5), A(u16), ACTF.Copy, scale=0.2))
                q5 = g16(nm("q5", blk))
                E(lambda: vec.tensor_tensor(A(q5), A(mu16), A(u5), ALU.mult))
                rel(nm("u5", blk))
                qf = g16(nm("qf", blk))
                E(lambda: act.activation(A(qf), A(q5), ACTF.Copy, scale=4.0))
                p16 = g16(nm("p16", blk))
                E(lambda: vec.tensor_tensor(A(p16), A(E4), A(q5),
                                            ALU.subtract))
                rel(nm("q5", blk))
                Fm16 = g16(nm("Fm16", blk))
                E(lambda: gps.tensor_tensor(A(Fm16), A(E4), A(qf), ALU.add))
                rel(nm("E4", blk), nm("qf", blk))
                irs16 = g16(nm("irs16", blk))
                E(lambda: vec.tensor_tensor(A(irs16), A(rinv16), A(sq16),
                                            ALU.mult))
                rel(nm("rinv", blk), nm("rinv16", blk))
                sqp16 = g16(nm("sqp16", blk))
                E(lambda: act.activation(A(sqp16), A(p16), ACTF.Sqrt,
                                         scale=float(GAMMA)))
                Ep16 = g16(nm("Ep16", blk))
                E(lambda: vec.tensor_tensor(A(Ep16), A(En16), A(p16),
                                            ALU.add))
                rel(nm("En16", blk))
                cc16 = g16(nm("cc16", blk))
                E(lambda: vec.tensor_tensor(A(cc16), A(sqp16), A(irs16),
                                            ALU.mult))
                rel(nm("sqp16", blk))
                sH16 = g16(nm("sH16", blk))
                E(lambda: vec.tensor_tensor(A(sH16), A(Ep16), A(irs16),
                                            ALU.mult))
                rel(nm("irs16", blk))
                su16 = g16(nm("su16", blk))
                E(lambda: gps.tensor_tensor(A(su16), A(sq16), A(u16),
                                            ALU.mult))
                Fe16 = g16(nm("Fe16", blk))
                E(lambda: gps.tensor_tensor(A(Fe16), A(u16), A(Ep16),
                                            ALU.mult))
                rel(nm("Ep16", blk))
                au16 = g16(nm("au16", blk))
                E(lambda: act.activation(A(au16), A(u16), ACTF.Abs))
                wsc16 = g16(nm("wsc16", blk))
                wm = wmaxL if blk == 0 else wmaxR
                E(lambda: (gps if CFG['wsc_pool'] else vec).tensor_tensor(
                    A(wsc16), A(au16), A(cc16), ALU.add))
                E(lambda: vec.tensor_reduce(wm[:], A(wsc16),
                                            axis=mybir.AxisListType.X,
                                            op=ALU.max))
                rel(nm("au16", blk), nm("cc16", blk), nm("wsc16", blk))
                return ops

            def emit_dt_launch(s):
                ci, co, gp = cc_in[s % 2], cc_out[s % 2], gppb[s % 2]
                return [
                    lambda: vec.tensor_tensor(wmax[:], wmaxL[:], wmaxR[:],
                                              ALU.max),
                    lambda: gps.partition_all_reduce(
                        gball[:], wmax[:], channels=P,
                        reduce_op=bass_isa.ReduceOp.max),
                    lambda: nc.sync.dma_start(out=ci[:],
                                              in_=gball[0:1, 0:1]),
                    lambda: gps.collective_compute(
                        "AllGather", ALU.bypass,
                        replica_groups=[list(range(NC))],
                        ins=[ci[:]], outs=[co[:]]),
                    lambda: nc.sync.dma_start(out=gp[:], in_=co[:]),
                ]

            def emit_dt_finish(s):
                gp = gppb[s % 2]
                return [
                    lambda: vec.tensor_reduce(g1[:], gp[:],
                                              axis=mybir.AxisListType.X,
                                              op=ALU.max),
                    lambda: gps.partition_broadcast(gb2[:], g1[:]),
                    lambda: vec.reciprocal_approx_fast(rgi[:], gb2[:]),
                    lambda: vec.tensor_scalar_mul(dt0[:], rgi[:],
                                                  float(CFL * DX)),
                    lambda: vec.scalar_tensor_tensor(rem[:], tcur[:], -1.0,
                                                     tfb[:], ALU.mult,
                                                     ALU.add),
                    lambda: vec.tensor_scalar_max(rem[:], rem[:], 0.0),
                    lambda: vec.tensor_tensor(dtt[:], dt0[:], rem[:], ALU.min),
                    lambda: vec.tensor_tensor(tcur[:], tcur[:], dtt[:],
                                              ALU.add),
                    lambda: vec.tensor_scalar_mul(hdtn[:], dtt[:],
                                                  float(-0.5 / DX)),
                ]

            def emit_B(s, blk):
                s = min(s, G - 1)
                ilo = s if blk == 0 else M - s - 1
                ihi = M - s - 1 if blk == 0 else W - s - 1

                def I(t):
                    return t[:, ilo:ihi]

                def Ls(t):
                    return t[:, ilo:ihi]

                def Rs(t):
                    return t[:, ilo + 1:ihi + 1]

                b = blk

                def R(*names):
                    rel(*(nm(x, b) for x in names))

                rho16 = live[nm("rho16", b)][0]
                p16 = live[nm("p16", b)][0]
                u16 = live[nm("u16", b)][0]
                sq16 = live[nm("sq16", b)][0]
                su16 = live[nm("su16", b)][0]
                sH16 = live[nm("sH16", b)][0]

                ops = []
                E = ops.append

                den32 = g32(nm("den32", b))
                E(lambda: (gps if CFG['den32_pool'] else vec).tensor_tensor(
                    I(den32), Ls(sq16), Rs(sq16), ALU.add))
                R("sq16")
                dinv32 = g32(nm("dinv32", b))
                E(lambda: vec.reciprocal_approx_fast(I(dinv32), I(den32)))
                R("den32")
                dinv16 = g16(nm("dinv16", b))
                E(lambda: act.copy(I(dinv16), I(dinv32)))
                R("dinv32")
                t16 = g16(nm("t16", b))
                E(lambda: vec.tensor_tensor(I(t16), Ls(su16), Rs(su16),
                                            ALU.add))
                R("su16")
                ur16 = g16(nm("ur16", b))
                E(lambda: vec.tensor_tensor(I(ur16), I(t16), I(dinv16),
                                            ALU.mult))
                R("t16")
                s16 = g16(nm("s16", b))
                E(lambda: vec.tensor_tensor(I(s16), Ls(sH16), Rs(sH16),
                                            ALU.add))
                R("sH16")
                Hr16 = g16(nm("Hr16", b))
                E(lambda: vec.tensor_tensor(I(Hr16), I(s16), I(dinv16),
                                            ALU.mult))
                R("s16", "dinv16")
                ur2 = g16(nm("ur2", b))
                E(lambda: act.activation(I(ur2), I(ur16), ACTF.Square))
                # d2 = 2c^2 = 0.8*Hr - 0.4*ur2 ; rd = 1/(2c^2)
                Hr8 = g16(nm("Hr8", b))
                E(lambda: act.activation(I(Hr8), I(Hr16), ACTF.Copy,
                                         scale=0.8))
                uh4 = g16(nm("uh4", b))
                E(lambda: act.activation(I(uh4), I(ur2), ACTF.Copy,
                                         scale=0.4))
                d32 = g32(nm("d32", b))
                E(lambda: (gps if CFG['den32_pool'] else vec).tensor_tensor(
                    I(d32), I(Hr8), I(uh4), ALU.subtract))
                R("Hr8", "uh4")
                cr16 = g16(nm("cr16", b))
                E(lambda: act.activation(I(cr16), I(d32), ACTF.Sqrt,
                                         scale=0.5))
                rd32 = g32(nm("rd32", b))
                E(lambda: vec.reciprocal_approx_fast(I(rd32), I(d32)))
                e2 = g16(nm("e2", b))
                E(lambda: act.activation(I(e2), I(d32), ACTF.Copy,
                                         scale=0.005))
                R("d32")
                rd16 = g16(nm("rd16", b))
                E(lambda: act.copy(I(rd16), I(rd32)))
                R("rd32")
                l1 = g16(nm("l1", b))
                E(lambda: vec.tensor_tensor(I(l1), I(ur16), I(cr16),
                                            ALU.subtract))
                l3 = g16(nm("l3", b))
                E(lambda: vec.tensor_tensor(I(l3), I(ur16), I(cr16), ALU.add))
                q1s = g16(nm("q1s", b))
                E(lambda: act.activation(I(q1s), I(l1), ACTF.Square))
                R("l1")
                q3s = g16(nm("q3s", b))
                E(lambda: act.activation(I(q3s), I(l3), ACTF.Square))
                R("l3")
                q1 = g16(nm("q1", b))
                E(lambda: vec.tensor_tensor(I(q1), I(q1s), I(e2), ALU.add))
                R("q1s")
                q3 = g16(nm("q3", b))
                E(lambda: vec.tensor_tensor(I(q3), I(q3s), I(e2), ALU.add))
                R("q3s")
                a2t = g16(nm("a2t", b))
                E(lambda: vec.tensor_tensor(I(a2t), I(ur2), I(e2), ALU.add))
                R("e2")
                a1 = g16(nm("a1", b))
                E(lambda: act.activation(I(a1), I(q1), ACTF.Sqrt))
                R("q1")
                a2 = g16(nm("a2", b))
                E(lambda: act.activation(I(a2), I(a2t), ACTF.Sqrt))
                R("a2t")
                a3 = g16(nm("a3", b))
                E(lambda: act.activation(I(a3), I(q3), ACTF.Sqrt))
                R("q3")
                drho = g16(nm("drho", b))
                E(lambda: (gps if CFG['drho_pool'] else vec).tensor_tensor(
                    I(drho), Rs(rho16), Ls(rho16), ALU.subtract))
                dp = g16(nm("dp", b))
                E(lambda: vec.tensor_tensor(I(dp), Rs(p16), Ls(p16),
                                            ALU.subtract))
                du = g16(nm("du", b))
                E(lambda: vec.tensor_tensor(I(du), Rs(u16), Ls(u16),
                                            ALU.subtract))
                w16 = g16(nm("w16", b))
                E(lambda: vec.tensor_tensor(I(w16), Rs(rho16), I(du),
                                            ALU.mult))
                R("du")
                crdu = g16(nm("crdu", b))
                E(lambda: vec.tensor_tensor(I(crdu), I(cr16), I(w16),
                                            ALU.mult))
                R("w16")
                x1 = g16(nm("x1", b))
                E(lambda: vec.tensor_tensor(I(x1), I(dp), I(crdu),
                                            ALU.subtract))
                x3 = g16(nm("x3", b))
                E(lambda: vec.tensor_tensor(I(x3), I(dp), I(crdu), ALU.add))
                R("crdu")
                y1 = g16(nm("y1", b))
                E(lambda: vec.tensor_tensor(I(y1), I(a1), I(x1), ALU.mult))
                R("a1", "x1")
                y3 = g16(nm("y3", b))
                E(lambda: vec.tensor_tensor(I(y3), I(a3), I(x3), ALU.mult))
                R("a3", "x3")
                bp = g16(nm("bp", b))
                E(lambda: vec.tensor_tensor(I(bp), I(y1), I(y3), ALU.add))
                bm = g16(nm("bm", b))
                E(lambda: vec.tensor_tensor(I(bm), I(y3), I(y1),
                                            ALU.subtract))
                R("y1", "y3")
                # m = 2*dp/(2c^2) = dp2 * rd
                dp2 = g16(nm("dp2", b))
                E(lambda: act.activation(I(dp2), I(dp), ACTF.Copy, scale=2.0))
                R("dp")
                m16 = g16(nm("m16", b))
                E(lambda: vec.tensor_tensor(I(m16), I(dp2), I(rd16),
                                            ALU.mult))
                R("dp2")
                al2 = g16(nm("al2", b))
                E(lambda: vec.tensor_tensor(I(al2), I(drho), I(m16),
                                            ALU.subtract))
                R("drho", "m16")
                G2 = g16(nm("G2", b))
                E(lambda: vec.tensor_tensor(I(G2), I(a2), I(al2), ALU.mult))
                R("a2", "al2")
                Sp = g16(nm("Sp", b))
                E(lambda: vec.tensor_tensor(I(Sp), I(bp), I(rd16), ALU.mult))
                R("bp")
                Sm = g16(nm("Sm", b))
                E(lambda: vec.tensor_tensor(I(Sm), I(bm), I(rd16), ALU.mult))
                R("bm", "rd16")
                dr = g16(nm("dr", b))
                E(lambda: vec.tensor_tensor(I(dr), I(Sp), I(G2), ALU.add))
                csm = g16(nm("csm", b))
                E(lambda: vec.tensor_tensor(I(csm), I(cr16), I(Sm), ALU.mult))
                R("cr16", "Sm")
                t2 = g16(nm("t2", b))
                E(lambda: vec.tensor_tensor(I(t2), I(ur16), I(dr), ALU.mult))
                dm = g16(nm("dm", b))
                E(lambda: vec.tensor_tensor(I(dm), I(t2), I(csm), ALU.add))
                R("t2")
                h2 = g16(nm("h2", b))
                E(lambda: act.activation(I(h2), I(ur2), ACTF.Copy, scale=0.5))
                R("ur2")
                z1 = g16(nm("z1", b))
                E(lambda: vec.tensor_tensor(I(z1), I(Hr16), I(Sp), ALU.mult))
                R("Hr16", "Sp")
                z2 = g16(nm("z2", b))
                E(lambda: vec.tensor_tensor(I(z2), I(h2), I(G2), ALU.mult))
                R("h2", "G2")
                z3 = g16(nm("z3", b))
                E(lambda: vec.tensor_tensor(I(z3), I(ur16), I(csm), ALU.mult))
                R("ur16", "csm")
                zz = g16(nm("zz", b))
                E(lambda: vec.tensor_tensor(I(zz), I(z1), I(z2), ALU.add))
                R("z1", "z2")
                de = g16(nm("de", b))
                E(lambda: vec.tensor_tensor(I(de), I(zz), I(z3), ALU.add))
                R("zz", "z3")
                return ops

            def emit_FU_flux(s, blk):
                s = min(s, G - 1)
                ilo = s if blk == 0 else M - s - 1
                ihi = M - s - 1 if blk == 0 else W - s - 1

                def I(t):
                    return t[:, ilo:ihi]

                def Ls(t):
                    return t[:, ilo:ihi]

                def Rs(t):
                    return t[:, ilo + 1:ihi + 1]

                b = blk
                mu16 = live[nm("mu16", b)][0]
                Fm16 = live[nm("Fm16", b)][0]
                Fe16 = live[nm("Fe16", b)][0]
                dr = live[nm("dr", b)][0]
                dm = live[nm("dm", b)][0]
                de = live[nm("de", b)][0]

                ops = []
                E = ops.append

                PrC = g16(nm("PrC", b))
                E(lambda: gps.tensor_tensor(I(PrC), Ls(mu16), Rs(mu16),
                                            ALU.add))
                E(lambda: vec.tensor_tensor(I(Pr), I(PrC), I(dr),
                                            ALU.subtract))
                PmC = g16(nm("PmC", b))
                E(lambda: gps.tensor_tensor(I(PmC), Ls(Fm16), Rs(Fm16),
                                            ALU.add))
                E(lambda: vec.tensor_tensor(I(Pm), I(PmC), I(dm),
                                            ALU.subtract))
                PeC = g16(nm("PeC", b))
                E(lambda: gps.tensor_tensor(I(PeC), Ls(Fe16), Rs(Fe16),
                                            ALU.add))
                E(lambda: vec.tensor_tensor(I(Pe), I(PeC), I(de),
                                            ALU.subtract))
                rel(nm("PrC", b), nm("PmC", b), nm("PeC", b),
                    nm("dr", b), nm("dm", b), nm("de", b),
                    nm("mu16", b), nm("Fm16", b), nm("Fe16", b),
                    nm("rho16", b), nm("p16", b), nm("u16", b))
                return ops

            def emit_FU_upd(s, blk):
                s = min(s, G - 1)
                ulo = s + 1 if blk == 0 else M - s - 1
                uhi = M - s - 1 if blk == 0 else W - s - 1
                un = uhi - ulo

                b = blk
                ops = []
                E = ops.append
                for Phi, st in ((Pr, rho), (Pm, mu), (Pe, En)):
                    dPhi = g16(nm("dPhi", b) + Phi.name)
                    E(lambda Phi=Phi, dPhi=dPhi: (
                        gps if CFG['dphi_pool'] else vec).tensor_tensor(
                        dPhi[:, ulo:uhi], Phi[:, ulo:ulo + un],
                        Phi[:, ulo - 1:ulo - 1 + un], ALU.subtract))
                    if CFG['upd_split']:
                        tmp = g16(nm("utmp", b) + Phi.name)
                        E(lambda dPhi=dPhi, tmp=tmp: vec.tensor_scalar(
                            tmp[:, ulo:uhi], dPhi[:, ulo:uhi], hdtn[:],
                            None, ALU.mult))
                        E(lambda st=st, tmp=tmp: gps.tensor_tensor(
                            st[:, ulo:uhi], st[:, ulo:uhi],
                            tmp[:, ulo:uhi], ALU.add))
                        rel(nm("utmp", b) + Phi.name)
                    else:
                        E(lambda st=st, dPhi=dPhi: vec.scalar_tensor_tensor(
                            st[:, ulo:uhi], dPhi[:, ulo:uhi], hdtn[:],
                            st[:, ulo:uhi], ALU.mult, ALU.add))
                    rel(nm("dPhi", b) + Phi.name)
                return ops

            def merge(*lists):
                idx = [0] * len(lists)
                remaining = sum(len(l) for l in lists)
                while remaining:
                    for i, l in enumerate(lists):
                        if idx[i] < len(l):
                            l[idx[i]]()
                            idx[i] += 1
                            remaining -= 1

            # ---- main loop ----
            tail = []
            for s in range(n_steps):
                pre = emit_clamps(s, 0) + emit_A(s, 0)
                merge(tail, pre)
                flush_pending()
                defer[0] = True
                aR = emit_clamps(s, 1) + emit_A(s, 1)
                defer[0] = False
                bL = emit_B(s, 0)
                merge(aR, bL)
                # dt launch at the merge boundary: all B(L) Pool work is
                # queued before the collective trigger
                for f in emit_dt_launch(s):
                    f()
                flush_pending()
                defer[0] = True
                fuLf = emit_FU_flux(s, 0)
                defer[0] = False
                bR = emit_B(s, 1)
                merge(bR, fuLf)
                for f in emit_dt_finish(s):
                    f()
                flush_pending()
                defer[0] = True
                tail = emit_FU_upd(s, 0) + emit_FU_flux(s, 1) \
                    + emit_FU_upd(s, 1)
                defer[0] = False
            merge(tail, [])
            flush_pending()

            assert len(free16) == N16 and len(free32) == N32, (
                len(free16), len(free32), list(live))

            # ---- epilogue ----
            own = slice(G, G + FPC)
            erinv = g32("erinv")
            vec.reciprocal_approx_fast(erinv[:, own], rho[:, own])
            uo = g32("euo")
            vec.tensor_tensor(uo[:, own], mu[:, own], erinv[:, own], ALU.mult)
            qo = g32("eqo")
            vec.tensor_tensor(qo[:, own], mu[:, own], uo[:, own], ALU.mult)
            E4o = g32("eE4o")
            vec.tensor_scalar_mul(E4o[:, own], En[:, own], 0.4)
            po = g32("epo")
            vec.scalar_tensor_tensor(po[:, own], qo[:, own], -0.2,
                                     E4o[:, own], ALU.mult, ALU.add)
            nc.sync.dma_start(out=rho_out.ap(), in_=rho[:, own])
            nc.sync.dma_start(out=u_out.ap(), in_=uo[:, own])
            nc.sync.dma_start(out=p_out.ap(), in_=po[:, own])

    nc.compile()
    return nc


def _get_program(n_steps: int):
    if n_steps not in _CACHE:
        _CACHE[n_steps] = _build(n_steps)
    return _CACHE[n_steps]


def make_in_maps(rho_init, u_init, p_init, tf):
    gm1 = np.float32(GAMMA - 1.0)
    cells = NX // NC
    idx = (np.arange(P)[:, None] * FPC) + (np.arange(W)[None, :] - G)
    in_maps = []
    for k in range(NC):
        gi = np.clip(k * cells + idx, 0, NX - 1)
        r = rho_init[gi]
        u = u_init[gi]
        p = p_init[gi]
        mus = r * u
        E = p / gm1 + np.float32(0.5) * r * u * u
        mskLa = np.zeros((P, G), np.uint8)
        mskRa = np.zeros((P, G), np.uint8)
        if k == 0:
            mskLa[0, :] = 1
        if k == NC - 1:
            mskRa[P - 1, :] = 1
        in_maps.append({
            "rho_in": np.ascontiguousarray(r),
            "mu_in": np.ascontiguousarray(mus),
            "E_in": np.ascontiguousarray(E),
            "tf_in": np.full((1, 1), tf, np.float32),
            "mskL_in": mskLa,
            "mskR_in": mskRa,
        })
    return in_maps


def kernel(rho_init, u_init, p_init, t_final, n_steps):
    rho_init = np.ascontiguousarray(np.asarray(rho_init, np.float32))
    u_init = np.ascontiguousarray(np.asarray(u_init, np.float32))
    p_init = np.ascontiguousarray(np.asarray(p_init, np.float32))
    tf = np.float32(np.asarray(t_final).reshape(()))
    ns = int(np.asarray(n_steps).reshape(()))
    assert rho_init.shape == (NX,)
    assert ns <= G

    in_maps = make_in_maps(rho_init, u_init, p_init, tf)
    nc = _get_program(ns)
    res = run_bass_kernel_spmd(nc, in_maps, core_ids=list(range(NC)))
    global _last_results
    _last_results = res

    cells = NX // NC
    rho_o = np.empty(NX, np.float32)
    u_o = np.empty(NX, np.float32)
    p_o = np.empty(NX, np.float32)
    for k in range(NC):
        sl = slice(k * cells, (k + 1) * cells)
        rho_o[sl] = res.results[k]["rho_out"].reshape(-1)
        u_o[sl] = res.results[k]["u_out"].reshape(-1)
        p_o[sl] = res.results[k]["p_out"].reshape(-1)
    return rho_o, u_o, p_o



# revision 3
# speedup vs baseline: 1.5052x; 1.5052x over previous
"""Trainium2 Bass kernel v4: fp16 compute, 3-engine balance, 2-way column
block split, STT-free Pool work.

Key structure:
  - 8 cores x [128 x 1024] cells + G=32 ghost columns per side; no halo
    exchange (ghosts absorb the 32-step stencil growth); shrinking valid
    windows replace ghost sanitization.
  - fp16 compute tiles (DVE 2x_1p); fp32 state, reciprocals, dt chain.
  - Two column blocks with a seam that drifts left one column per step:
    cross-step deps stay within a block, seam deps point only L -> R.
    Block op streams are round-robin merged so each in-order engine queue
    alternates blocks and a stalled op is followed by ready work.
  - Pool (GpSimd) gets only plain TensorTensor ops (neuronxcc rejects
    TensorScalarPtr on Pool).  All former scalar_tensor_tensor fusions are
    decomposed into ACT scalar-scale ops + plain TTs; 1/(2c^2) comes
    straight from recip(2c^2).
  - The dt AllReduce is launched at a merge boundary (after all B(L) Pool
    work) and finished after B(R), so the collective never blocks the Pool
    queue head.
"""

import numpy as np

import concourse.bass as bass
import concourse.bacc as bacc
import concourse.tile as tile
import concourse.mybir as mybir
from concourse import bass_isa
from concourse.bass_utils import run_bass_kernel_spmd

F32 = mybir.dt.float32
F16 = mybir.dt.float16
U8 = mybir.dt.uint8
ALU = mybir.AluOpType
ACTF = mybir.ActivationFunctionType

GAMMA = 1.4
CFL = 0.5
DX = 1e-3

NX = 1048576
NC = 8
P = 128
FPC = NX // NC // P          # 1024 cells per partition
G = 32                       # ghost width per side (>= n_steps)
W = FPC + 2 * G              # 1088 columns per partition
M = W // 2                   # block seam at step 0 (drifts left)

_CACHE = {}
_last_results = None


def _build(n_steps: int):
    nc = bacc.Bacc("TRN2", target_bir_lowering=False, debug=False,
                   enable_asserts=False, num_devices=NC)

    rho_in = nc.dram_tensor("rho_in", [P, W], F32, kind="ExternalInput")
    mu_in = nc.dram_tensor("mu_in", [P, W], F32, kind="ExternalInput")
    E_in = nc.dram_tensor("E_in", [P, W], F32, kind="ExternalInput")
    tf_in = nc.dram_tensor("tf_in", [1, 1], F32, kind="ExternalInput")
    mskL_in = nc.dram_tensor("mskL_in", [P, G], U8, kind="ExternalInput")
    mskR_in = nc.dram_tensor("mskR_in", [P, G], U8, kind="ExternalInput")
    rho_out = nc.dram_tensor("rho_out", [P, FPC], F32, kind="ExternalOutput")
    u_out = nc.dram_tensor("u_out", [P, FPC], F32, kind="ExternalOutput")
    p_out = nc.dram_tensor("p_out", [P, FPC], F32, kind="ExternalOutput")

    with tile.TileContext(nc) as tc:
        with (
            tc.tile_pool(name="sb", bufs=1) as sb,
            tc.tile_pool(name="dram", bufs=1, space="DRAM") as dram,
        ):
            rho = sb.tile([P, W], F32, tag="rho", name="rho")
            mu = sb.tile([P, W], F32, tag="mu", name="mu")
            En = sb.tile([P, W], F32, tag="En", name="En")

            N32 = 8
            N16 = 44
            wk32 = [sb.tile([P, W], F32, tag=f"w32_{i}", name=f"w32_{i}")
                    for i in range(N32)]
            wk16 = [sb.tile([P, W], F16, tag=f"w16_{i}", name=f"w16_{i}")
                    for i in range(N16)]
            free32 = list(wk32)
            free16 = list(wk16)
            live = {}
            pending = []
            defer = [False]
            def g32(name):
                t = free32.pop()
                live[name] = (t, free32)
                return t

            def g16(name):
                t = free16.pop()
                live[name] = (t, free16)
                return t

            def rel(*names):
                # Tiles freed while constructing the FIRST stream of a merge
                # pair must not be re-allocated by the second stream (their
                # thunks interleave at emission), so those parks go to
                # `pending` until the merge completes.
                for n in names:
                    t, pool = live.pop(n)
                    if defer[0]:
                        pending.append((t, pool))
                    else:
                        pool.append(t)

            def flush_pending():
                for t, pool in pending:
                    pool.append(t)
                pending.clear()

            mskL = sb.tile([P, G], U8, tag="mskL", name="mskL")
            mskR = sb.tile([P, G], U8, tag="mskR", name="mskR")
            wmaxL = sb.tile([P, 1], F32, tag="wmaxL", name="wmaxL")
            wmaxR = sb.tile([P, 1], F32, tag="wmaxR", name="wmaxR")
            wmax = sb.tile([P, 1], F32, tag="wmax", name="wmax")
            gpp = sb.tile([P, 1], F32, tag="gpp", name="gpp")
            gball = sb.tile([P, 1], F32, tag="gball", name="gball")
            rgi = sb.tile([P, 1], F32, tag="rgi", name="rgi")
            dt0 = sb.tile([P, 1], F32, tag="dt0", name="dt0")
            rem = sb.tile([P, 1], F32, tag="rem", name="rem")
            dtt = sb.tile([P, 1], F32, tag="dtt", name="dtt")
            tcur = sb.tile([P, 1], F32, tag="tcur", name="tcur")
            hdtn = sb.tile([P, 1], F32, tag="hdtn", name="hdtn")
            tf1 = sb.tile([1, 1], F32, tag="tf1", name="tf1")
            tfb = sb.tile([P, 1], F32, tag="tfb", name="tfb")

            cc_in = [dram.tile([P, 1], F32, tag=f"cc_in{i}",
                                name=f"cc_in{i}") for i in range(2)]
            cc_out = [dram.tile([P, 1], F32, tag=f"cc_out{i}",
                                 name=f"cc_out{i}") for i in range(2)]
            gppb = [sb.tile([P, 1], F32, tag=f"gpp{i}", name=f"gpp{i}")
                    for i in range(2)]

            vec = nc.vector
            act = nc.scalar
            gps = nc.gpsimd

            # full-width shared flux tiles (the update diff at the seam
            # reads across blocks)
            Pr = sb.tile([P, W], F16, tag="Pr", name="Pr")
            Pm = sb.tile([P, W], F16, tag="Pm", name="Pm")
            Pe = sb.tile([P, W], F16, tag="Pe", name="Pe")

            # ---- prologue ----
            nc.sync.dma_start(out=rho[:], in_=rho_in.ap())
            nc.sync.dma_start(out=mu[:], in_=mu_in.ap())
            nc.sync.dma_start(out=En[:], in_=E_in.ap())
            nc.sync.dma_start(out=mskL[:], in_=mskL_in.ap())
            nc.sync.dma_start(out=mskR[:], in_=mskR_in.ap())
            nc.sync.dma_start(out=tf1[:], in_=tf_in.ap())
            gps.partition_broadcast(tfb[:], tf1[:])
            vec.memset(tcur[:], 0.0)

            def nm(base, blk):
                return f"{base}@{blk}"

            def emit_clamps(s, blk):
                ops = []
                if s == 0 or s >= G:
                    return ops
                gw = G - s
                if blk == 0:
                    for st in (rho, mu, En):
                        ops.append(lambda st=st: vec.copy_predicated(
                            st[:, s:G], mskL[:, 0:gw],
                            st[:, G:G + 1].broadcast_to((P, gw))))
                else:
                    for st in (rho, mu, En):
                        ops.append(lambda st=st: vec.copy_predicated(
                            st[:, W - G:W - s], mskR[:, 0:gw],
                            st[:, W - G - 1:W - G].broadcast_to((P, gw))))
                return ops

            # Block ranges (seam at M - s):
            #   A0 = [s, M-s)        A1 = [M-s, W-s)
            #   I0 = [s, M-s-1)      I1 = [M-s-1, W-s-1)
            #   U0 = [s+1, M-s-1)    U1 = [M-s-1, W-s-1)
            def emit_A(s, blk):
                # block R overlaps one column left of the seam so its B-phase
                # reads only its own stage-A tiles
                s = min(s, G - 1)
                lo = s if blk == 0 else M - s - 1
                hi = M - s if blk == 0 else W - s

                def A(t):
                    return t[:, lo:hi]

                ops = []
                E = ops.append

                rho16 = g16(nm("rho16", blk))
                mu16 = g16(nm("mu16", blk))
                En16 = g16(nm("En16", blk))
                E(lambda: act.copy(A(rho16), A(rho)))
                E(lambda: act.copy(A(mu16), A(mu)))
                rinv = g32(nm("rinv", blk))
                E(lambda: vec.reciprocal_approx_fast(A(rinv), A(rho)))
                u16 = g16(nm("u16", blk))
                E(lambda: vec.tensor_tensor(A(u16), A(mu), A(rinv), ALU.mult))
                sq16 = g16(nm("sq16", blk))
                E(lambda: act.activation(A(sq16), A(rho), ACTF.Sqrt))
                E4 = g16(nm("E4", blk))
                E(lambda: act.activation(A(E4), A(En), ACTF.Copy, scale=0.4))
                E(lambda: act.copy(A(En16), A(En)))
                # p = E4 - 0.2*q, Fm = E4 + 0.8*q  via q5 = mu*(0.2 u)
                u5 = g16(nm("u5", blk))
                E(lambda: act.activation(A(u5), A(u16), ACTF.Copy, scale=0.2))
                q5 = g16(nm("q5", blk))
                E(lambda: vec.tensor_tensor(A(q5), A(mu16), A(u5), ALU.mult))
                rel(nm("u5", blk))
                qf = g16(nm("qf", blk))
                E(lambda: act.activation(A(qf), A(q5), ACTF.Copy, scale=4.0))
                p16 = g16(nm("p16", blk))
                E(lambda: vec.tensor_tensor(A(p16), A(E4), A(q5),
                                            ALU.subtract))
                rel(nm("q5", blk))
                Fm16 = g16(nm("Fm16", blk))
                E(lambda: gps.tensor_tensor(A(Fm16), A(E4), A(qf), ALU.add))
                rel(nm("E4", blk), nm("qf", blk))
                irs16 = g16(nm("irs16", blk))
                E(lambda: vec.tensor_tensor(A(irs16), A(rinv), A(sq16),
                                            ALU.mult))
                rel(nm("rinv", blk))
                sqp16 = g16(nm("sqp16", blk))
                E(lambda: act.activation(A(sqp16), A(p16), ACTF.Sqrt,
                                         scale=float(GAMMA)))
                Ep16 = g16(nm("Ep16", blk))
                E(lambda: vec.tensor_tensor(A(Ep16), A(En16), A(p16),
                                            ALU.add))
                rel(nm("En16", blk))
                cc16 = g16(nm("cc16", blk))
                E(lambda: vec.tensor_tensor(A(cc16), A(sqp16), A(irs16),
                                            ALU.mult))
                rel(nm("sqp16", blk))
                sH16 = g16(nm("sH16", blk))
                E(lambda: vec.tensor_tensor(A(sH16), A(Ep16), A(irs16),
                                            ALU.mult))
                rel(nm("irs16", blk))
                su16 = g16(nm("su16", blk))
                E(lambda: gps.tensor_tensor(A(su16), A(sq16), A(u16),
                                            ALU.mult))
                Fe16 = g16(nm("Fe16", blk))
                E(lambda: gps.tensor_tensor(A(Fe16), A(u16), A(Ep16),
                                            ALU.mult))
                rel(nm("Ep16", blk))
                au16 = g16(nm("au16", blk))
                E(lambda: act.activation(A(au16), A(u16), ACTF.Abs))
                wsc16 = g16(nm("wsc16", blk))
                wm = wmaxL if blk == 0 else wmaxR
                E(lambda: vec.tensor_tensor(A(wsc16), A(au16), A(cc16),
                                            ALU.add))
                E(lambda: vec.tensor_reduce(wm[:], A(wsc16),
                                            axis=mybir.AxisListType.X,
                                            op=ALU.max))
                rel(nm("au16", blk), nm("cc16", blk), nm("wsc16", blk))
                return ops

            def emit_dt_launch(s):
                ci, co, gp = cc_in[s % 2], cc_out[s % 2], gppb[s % 2]
                return [
                    lambda: vec.tensor_tensor(wmax[:], wmaxL[:], wmaxR[:],
                                              ALU.max),
                    lambda: nc.sync.dma_start(out=ci[:], in_=wmax[:]),
                    lambda: gps.collective_compute(
                        "AllReduce", ALU.max,
                        replica_groups=[list(range(NC))],
                        ins=[ci[:]], outs=[co[:]]),
                    lambda: nc.sync.dma_start(out=gp[:], in_=co[:]),
                ]

            def emit_dt_finish(s):
                gp = gppb[s % 2]
                return [
                    lambda: gps.partition_all_reduce(
                        gball[:], gp[:], channels=P,
                        reduce_op=bass_isa.ReduceOp.max),
                    lambda: vec.reciprocal_approx_fast(rgi[:], gball[:]),
                    lambda: vec.tensor_scalar_mul(dt0[:], rgi[:],
                                                  float(CFL * DX)),
                    lambda: vec.scalar_tensor_tensor(rem[:], tcur[:], -1.0,
                                                     tfb[:], ALU.mult,
                                                     ALU.add),
                    lambda: vec.tensor_scalar_max(rem[:], rem[:], 0.0),
                    lambda: vec.tensor_tensor(dtt[:], dt0[:], rem[:], ALU.min),
                    lambda: vec.tensor_tensor(tcur[:], tcur[:], dtt[:],
                                              ALU.add),
                    lambda: vec.tensor_scalar_mul(hdtn[:], dtt[:],
                                                  float(-0.5 / DX)),
                ]

            def emit_B(s, blk):
                s = min(s, G - 1)
                ilo = s if blk == 0 else M - s - 1
                ihi = M - s - 1 if blk == 0 else W - s - 1

                def I(t):
                    return t[:, ilo:ihi]

                def Ls(t):
                    return t[:, ilo:ihi]

                def Rs(t):
                    return t[:, ilo + 1:ihi + 1]

                b = blk

                def R(*names):
                    rel(*(nm(x, b) for x in names))

                rho16 = live[nm("rho16", b)][0]
                p16 = live[nm("p16", b)][0]
                u16 = live[nm("u16", b)][0]
                sq16 = live[nm("sq16", b)][0]
                su16 = live[nm("su16", b)][0]
                sH16 = live[nm("sH16", b)][0]

                ops = []
                E = ops.append

                den32 = g32(nm("den32", b))
                E(lambda: vec.tensor_tensor(I(den32), Ls(sq16), Rs(sq16),
                                            ALU.add))
                R("sq16")
                dinv32 = g32(nm("dinv32", b))
                E(lambda: vec.reciprocal_approx_fast(I(dinv32), I(den32)))
                R("den32")
                dinv16 = g16(nm("dinv16", b))
                E(lambda: act.copy(I(dinv16), I(dinv32)))
                R("dinv32")
                t16 = g16(nm("t16", b))
                E(lambda: vec.tensor_tensor(I(t16), Ls(su16), Rs(su16),
                                            ALU.add))
                R("su16")
                ur16 = g16(nm("ur16", b))
                E(lambda: vec.tensor_tensor(I(ur16), I(t16), I(dinv16),
                                            ALU.mult))
                R("t16")
                s16 = g16(nm("s16", b))
                E(lambda: vec.tensor_tensor(I(s16), Ls(sH16), Rs(sH16),
                                            ALU.add))
                R("sH16")
                Hr16 = g16(nm("Hr16", b))
                E(lambda: vec.tensor_tensor(I(Hr16), I(s16), I(dinv16),
                                            ALU.mult))
                R("s16", "dinv16")
                ur2 = g16(nm("ur2", b))
                E(lambda: act.activation(I(ur2), I(ur16), ACTF.Square))
                # d2 = 2c^2 = 0.8*Hr - 0.4*ur2 ; rd = 1/(2c^2)
                Hr8 = g16(nm("Hr8", b))
                E(lambda: act.activation(I(Hr8), I(Hr16), ACTF.Copy,
                                         scale=0.8))
                uh4 = g16(nm("uh4", b))
                E(lambda: act.activation(I(uh4), I(ur2), ACTF.Copy,
                                         scale=0.4))
                d32 = g32(nm("d32", b))
                E(lambda: vec.tensor_tensor(I(d32), I(Hr8), I(uh4),
                                            ALU.subtract))
                R("Hr8", "uh4")
                cr16 = g16(nm("cr16", b))
                E(lambda: act.activation(I(cr16), I(d32), ACTF.Sqrt,
                                         scale=0.5))
                rd32 = g32(nm("rd32", b))
                E(lambda: vec.reciprocal_approx_fast(I(rd32), I(d32)))
                e2 = g16(nm("e2", b))
                E(lambda: act.activation(I(e2), I(d32), ACTF.Copy,
                                         scale=0.005))
                R("d32")
                rd16 = g16(nm("rd16", b))
                E(lambda: act.copy(I(rd16), I(rd32)))
                R("rd32")
                l1 = g16(nm("l1", b))
                E(lambda: vec.tensor_tensor(I(l1), I(ur16), I(cr16),
                                            ALU.subtract))
                l3 = g16(nm("l3", b))
                E(lambda: vec.tensor_tensor(I(l3), I(ur16), I(cr16), ALU.add))
                q1s = g16(nm("q1s", b))
                E(lambda: act.activation(I(q1s), I(l1), ACTF.Square))
                R("l1")
                q3s = g16(nm("q3s", b))
                E(lambda: act.activation(I(q3s), I(l3), ACTF.Square))
                R("l3")
                q1 = g16(nm("q1", b))
                E(lambda: vec.tensor_tensor(I(q1), I(q1s), I(e2), ALU.add))
                R("q1s")
                q3 = g16(nm("q3", b))
                E(lambda: vec.tensor_tensor(I(q3), I(q3s), I(e2), ALU.add))
                R("q3s")
                a2t = g16(nm("a2t", b))
                E(lambda: vec.tensor_tensor(I(a2t), I(ur2), I(e2), ALU.add))
                R("e2")
                a1 = g16(nm("a1", b))
                E(lambda: act.activation(I(a1), I(q1), ACTF.Sqrt))
                R("q1")
                a2 = g16(nm("a2", b))
                E(lambda: act.activation(I(a2), I(a2t), ACTF.Sqrt))
                R("a2t")
                a3 = g16(nm("a3", b))
                E(lambda: act.activation(I(a3), I(q3), ACTF.Sqrt))
                R("q3")
                drho = g16(nm("drho", b))
                E(lambda: vec.tensor_tensor(I(drho), Rs(rho16), Ls(rho16),
                                            ALU.subtract))
                dp = g16(nm("dp", b))
                E(lambda: vec.tensor_tensor(I(dp), Rs(p16), Ls(p16),
                                            ALU.subtract))
                du = g16(nm("du", b))
                E(lambda: vec.tensor_tensor(I(du), Rs(u16), Ls(u16),
                                            ALU.subtract))
                w16 = g16(nm("w16", b))
                E(lambda: vec.tensor_tensor(I(w16), Rs(rho16), I(du),
                                            ALU.mult))
                R("du")
                crdu = g16(nm("crdu", b))
                E(lambda: vec.tensor_tensor(I(crdu), I(cr16), I(w16),
                                            ALU.mult))
                R("w16")
                x1 = g16(nm("x1", b))
                E(lambda: vec.tensor_tensor(I(x1), I(dp), I(crdu),
                                            ALU.subtract))
                x3 = g16(nm("x3", b))
                E(lambda: vec.tensor_tensor(I(x3), I(dp), I(crdu), ALU.add))
                R("crdu")
                y1 = g16(nm("y1", b))
                E(lambda: vec.tensor_tensor(I(y1), I(a1), I(x1), ALU.mult))
                R("a1", "x1")
                y3 = g16(nm("y3", b))
                E(lambda: vec.tensor_tensor(I(y3), I(a3), I(x3), ALU.mult))
                R("a3", "x3")
                bp = g16(nm("bp", b))
                E(lambda: vec.tensor_tensor(I(bp), I(y1), I(y3), ALU.add))
                bm = g16(nm("bm", b))
                E(lambda: vec.tensor_tensor(I(bm), I(y3), I(y1),
                                            ALU.subtract))
                R("y1", "y3")
                # m = 2*dp/(2c^2) = dp2 * rd
                dp2 = g16(nm("dp2", b))
                E(lambda: act.activation(I(dp2), I(dp), ACTF.Copy, scale=2.0))
                R("dp")
                m16 = g16(nm("m16", b))
                E(lambda: vec.tensor_tensor(I(m16), I(dp2), I(rd16),
                                            ALU.mult))
                R("dp2")
                al2 = g16(nm("al2", b))
                E(lambda: vec.tensor_tensor(I(al2), I(drho), I(m16),
                                            ALU.subtract))
                R("drho", "m16")
                G2 = g16(nm("G2", b))
                E(lambda: vec.tensor_tensor(I(G2), I(a2), I(al2), ALU.mult))
                R("a2", "al2")
                Sp = g16(nm("Sp", b))
                E(lambda: vec.tensor_tensor(I(Sp), I(bp), I(rd16), ALU.mult))
                R("bp")
                Sm = g16(nm("Sm", b))
                E(lambda: vec.tensor_tensor(I(Sm), I(bm), I(rd16), ALU.mult))
                R("bm", "rd16")
                dr = g16(nm("dr", b))
                E(lambda: vec.tensor_tensor(I(dr), I(Sp), I(G2), ALU.add))
                csm = g16(nm("csm", b))
                E(lambda: vec.tensor_tensor(I(csm), I(cr16), I(Sm), ALU.mult))
                R("cr16", "Sm")
                t2 = g16(nm("t2", b))
                E(lambda: vec.tensor_tensor(I(t2), I(ur16), I(dr), ALU.mult))
                dm = g16(nm("dm", b))
                E(lambda: vec.tensor_tensor(I(dm), I(t2), I(csm), ALU.add))
                R("t2")
                h2 = g16(nm("h2", b))
                E(lambda: act.activation(I(h2), I(ur2), ACTF.Copy, scale=0.5))
                R("ur2")
                z1 = g16(nm("z1", b))
                E(lambda: vec.tensor_tensor(I(z1), I(Hr16), I(Sp), ALU.mult))
                R("Hr16", "Sp")
                z2 = g16(nm("z2", b))
                E(lambda: vec.tensor_tensor(I(z2), I(h2), I(G2), ALU.mult))
                R("h2", "G2")
                z3 = g16(nm("z3", b))
                E(lambda: vec.tensor_tensor(I(z3), I(ur16), I(csm), ALU.mult))
                R("ur16", "csm")
                zz = g16(nm("zz", b))
                E(lambda: vec.tensor_tensor(I(zz), I(z1), I(z2), ALU.add))
                R("z1", "z2")
                de = g16(nm("de", b))
                E(lambda: vec.tensor_tensor(I(de), I(zz), I(z3), ALU.add))
                R("zz", "z3")
                return ops

            def emit_FU_flux(s, blk):
                s = min(s, G - 1)
                ilo = s if blk == 0 else M - s - 1
                ihi = M - s - 1 if blk == 0 else W - s - 1

                def I(t):
                    return t[:, ilo:ihi]

                def Ls(t):
                    return t[:, ilo:ihi]

                def Rs(t):
                    return t[:, ilo + 1:ihi + 1]

                b = blk
                mu16 = live[nm("mu16", b)][0]
                Fm16 = live[nm("Fm16", b)][0]
                Fe16 = live[nm("Fe16", b)][0]
                dr = live[nm("dr", b)][0]
                dm = live[nm("dm", b)][0]
                de = live[nm("de", b)][0]

                ops = []
                E = ops.append

                PrC = g16(nm("PrC", b))
                E(lambda: gps.tensor_tensor(I(PrC), Ls(mu16), Rs(mu16),
                                            ALU.add))
                E(lambda: vec.tensor_tensor(I(Pr), I(PrC), I(dr),
                                            ALU.subtract))
                PmC = g16(nm("PmC", b))
                E(lambda: gps.tensor_tensor(I(PmC), Ls(Fm16), Rs(Fm16),
                                            ALU.add))
                E(lambda: vec.tensor_tensor(I(Pm), I(PmC), I(dm),
                                            ALU.subtract))
                PeC = g16(nm("PeC", b))
                E(lambda: gps.tensor_tensor(I(PeC), Ls(Fe16), Rs(Fe16),
                                            ALU.add))
                E(lambda: vec.tensor_tensor(I(Pe), I(PeC), I(de),
                                            ALU.subtract))
                rel(nm("PrC", b), nm("PmC", b), nm("PeC", b),
                    nm("dr", b), nm("dm", b), nm("de", b),
                    nm("mu16", b), nm("Fm16", b), nm("Fe16", b),
                    nm("rho16", b), nm("p16", b), nm("u16", b))
                return ops

            def emit_FU_upd(s, blk):
                s = min(s, G - 1)
                ulo = s + 1 if blk == 0 else M - s - 1
                uhi = M - s - 1 if blk == 0 else W - s - 1
                un = uhi - ulo

                b = blk
                ops = []
                E = ops.append
                for Phi, st in ((Pr, rho), (Pm, mu), (Pe, En)):
                    dPhi = g16(nm("dPhi", b) + Phi.name)
                    E(lambda Phi=Phi, dPhi=dPhi: vec.tensor_tensor(
                        dPhi[:, ulo:uhi], Phi[:, ulo:ulo + un],
                        Phi[:, ulo - 1:ulo - 1 + un], ALU.subtract))
                    E(lambda st=st, dPhi=dPhi: vec.scalar_tensor_tensor(
                        st[:, ulo:uhi], dPhi[:, ulo:uhi], hdtn[:],
                        st[:, ulo:uhi], ALU.mult, ALU.add))
                    rel(nm("dPhi", b) + Phi.name)
                return ops

            def merge(*lists):
                idx = [0] * len(lists)
                remaining = sum(len(l) for l in lists)
                while remaining:
                    for i, l in enumerate(lists):
                        if idx[i] < len(l):
                            l[idx[i]]()
                            idx[i] += 1
                            remaining -= 1

            # ---- main loop ----
            tail = []
            for s in range(n_steps):
                pre = emit_clamps(s, 0) + emit_A(s, 0)
                merge(tail, pre)
                flush_pending()
                defer[0] = True
                aR = emit_clamps(s, 1) + emit_A(s, 1)
                defer[0] = False
                bL = emit_B(s, 0)
                merge(aR, bL)
                # dt launch at the merge boundary: all B(L) Pool work is
                # queued before the collective trigger
                for f in emit_dt_launch(s):
                    f()
                flush_pending()
                defer[0] = True
                fuLf = emit_FU_flux(s, 0)
                defer[0] = False
                bR = emit_B(s, 1)
                merge(bR, fuLf)
                for f in emit_dt_finish(s):
                    f()
                flush_pending()
                defer[0] = True
                tail = emit_FU_upd(s, 0) + emit_FU_flux(s, 1) \
                    + emit_FU_upd(s, 1)
                defer[0] = False
            merge(tail, [])
            flush_pending()

            assert len(free16) == N16 and len(free32) == N32, (
                len(free16), len(free32), list(live))

            # ---- epilogue ----
            own = slice(G, G + FPC)
            erinv = g32("erinv")
            vec.reciprocal_approx_fast(erinv[:, own], rho[:, own])
            uo = g32("euo")
            vec.tensor_tensor(uo[:, own], mu[:, own], erinv[:, own], ALU.mult)
            qo = g32("eqo")
            vec.tensor_tensor(qo[:, own], mu[:, own], uo[:, own], ALU.mult)
            E4o = g32("eE4o")
            vec.tensor_scalar_mul(E4o[:, own], En[:, own], 0.4)
            po = g32("epo")
            vec.scalar_tensor_tensor(po[:, own], qo[:, own], -0.2,
                                     E4o[:, own], ALU.mult, ALU.add)
            nc.sync.dma_start(out=rho_out.ap(), in_=rho[:, own])
            nc.sync.dma_start(out=u_out.ap(), in_=uo[:, own])
            nc.sync.dma_start(out=p_out.ap(), in_=po[:, own])

    nc.compile()
    return nc


def _get_program(n_steps: int):
    if n_steps not in _CACHE:
        _CACHE[n_steps] = _build(n_steps)
    return _CACHE[n_steps]


def make_in_maps(rho_init, u_init, p_init, tf):
    gm1 = np.float32(GAMMA - 1.0)
    cells = NX // NC
    idx = (np.arange(P)[:, None] * FPC) + (np.arange(W)[None, :] - G)
    in_maps = []
    for k in range(NC):
        gi = np.clip(k * cells + idx, 0, NX - 1)
        r = rho_init[gi]
        u = u_init[gi]
        p = p_init[gi]
        mus = r * u
        E = p / gm1 + np.float32(0.5) * r * u * u
        mskLa = np.zeros((P, G), np.uint8)
        mskRa = np.zeros((P, G), np.uint8)
        if k == 0:
            mskLa[0, :] = 1
        if k == NC - 1:
            mskRa[P - 1, :] = 1
        in_maps.append({
            "rho_in": np.ascontiguousarray(r),
            "mu_in": np.ascontiguousarray(mus),
            "E_in": np.ascontiguousarray(E),
            "tf_in": np.full((1, 1), tf, np.float32),
            "mskL_in": mskLa,
            "mskR_in": mskRa,
        })
    return in_maps


def kernel(rho_init, u_init, p_init, t_final, n_steps):
    rho_init = np.ascontiguousarray(np.asarray(rho_init, np.float32))
    u_init = np.ascontiguousarray(np.asarray(u_init, np.float32))
    p_init = np.ascontiguousarray(np.asarray(p_init, np.float32))
    tf = np.float32(np.asarray(t_final).reshape(()))
    ns = int(np.asarray(n_steps).reshape(()))
    assert rho_init.shape == (NX,)
    assert ns <= G

    in_maps = make_in_maps(rho_init, u_init, p_init, tf)
    nc = _get_program(ns)
    res = run_bass_kernel_spmd(nc, in_maps, core_ids=list(range(NC)))
    global _last_results
    _last_results = res

    cells = NX // NC
    rho_o = np.empty(NX, np.float32)
    u_o = np.empty(NX, np.float32)
    p_o = np.empty(NX, np.float32)
    for k in range(NC):
        sl = slice(k * cells, (k + 1) * cells)
        rho_o[sl] = res.results[k]["rho_out"].reshape(-1)
        u_o[sl] = res.results[k]["u_out"].reshape(-1)
        p_o[sl] = res.results[k]["p_out"].reshape(-1)
    return rho_o, u_o, p_o

